# revision 4
# baseline (speedup 1.0000x reference)
"""ViT block kernel for Trainium2, data-parallel over batch across 8 cores.

Per-core program (sequence of 1024 tokens, dim 768, 12 heads, mlp 3072),
everything kept in transposed [feature, seq] layout on device:

  LN1  : col-sums via ones-matmul on PE + rank-1 broadcast matmul
  QKV  : weights stationary (lhsT), activations moving
  attn : dots^T = K @ Q^T per head (K=64), exp on ACT (scale folded),
         attn@V with a ones-augmented V column -> softmax sums for free,
         division deferred to the [64, seq] head output
  Wo   : + residual (fp32)
  LN2  : same as LN1
  FFN  : relu(x@W1+b1)@W2 + b2, weights streamed as M-slabs
  out  : transposed output, un-transposed on host

Matmul operands in bf16 (fp32 PSUM accumulation); residual stream, softmax
sums, reciprocals, LN stats applications in fp32.  Host pre-transposes x,
pre-casts weights, and re-transposes the output.
"""

import numpy as np
import ml_dtypes

import concourse.bass as bass
from concourse import bacc
import concourse.mybir as mybir
import concourse.tile as tile
from concourse.bass import ts, ds
from concourse.bass_utils import run_bass_kernel_spmd

F32 = mybir.dt.float32
BF16 = mybir.dt.bfloat16
AF = mybir.ActivationFunctionType
ALU = mybir.AluOpType

# Problem shape (hardcoded; harness always uses this config)
B = 8          # batch == number of cores
N = 1024       # sequence length
D = 768        # model dim
H = 12         # heads
DH = 64        # head dim
F = 3072       # mlp dim
P = 128        # partitions
NT = N // P    # 8 seq tiles
DT = D // P    # 6 dim tiles
FT = F // P    # 24 mlp tiles
NCH = 512      # psum free-dim chunk
NC = N // NCH  # 2 chunks
SCALE = DH ** -0.5

# Matmul operand mode: "bf16" | "f32r" | "f32"
MM_MODE = "bf16"

if MM_MODE == "bf16":
    MM_SB = BF16            # SBUF dtype of matmul operands
    MM_NP = ml_dtypes.bfloat16
else:
    MM_SB = F32
    MM_NP = np.float32


def _mm(ap):
    """Wrap a matmul operand AP for the selected mode."""
    if MM_MODE == "f32r":
        return ap.bitcast(mybir.dt.float32r)
    return ap


def build_program():
    nc = bacc.Bacc("TRN2", target_bir_lowering=False)

    # ---- DRAM parameters ----
    xt = nc.dram_tensor("xt", [D, N], F32, kind="ExternalInput").ap()
    wq = nc.dram_tensor("wq", [D, D], MM_SB, kind="ExternalInput").ap()
    wk = nc.dram_tensor("wk", [D, D], MM_SB, kind="ExternalInput").ap()
    wv = nc.dram_tensor("wv", [D, D], MM_SB, kind="ExternalInput").ap()
    wo = nc.dram_tensor("wo", [D, D], MM_SB, kind="ExternalInput").ap()
    w1 = nc.dram_tensor("w1", [D, F], MM_SB, kind="ExternalInput").ap()
    w2 = nc.dram_tensor("w2", [F, D], MM_SB, kind="ExternalInput").ap()
    bo = nc.dram_tensor("bo", [D], F32, kind="ExternalInput").ap()
    b1 = nc.dram_tensor("b1", [F], F32, kind="ExternalInput").ap()
    b2 = nc.dram_tensor("b2", [D], F32, kind="ExternalInput").ap()
    ln1w = nc.dram_tensor("ln1w", [D], F32, kind="ExternalInput").ap()
    ln1b = nc.dram_tensor("ln1b", [D], F32, kind="ExternalInput").ap()
    ln2w = nc.dram_tensor("ln2w", [D], F32, kind="ExternalInput").ap()
    ln2b = nc.dram_tensor("ln2b", [D], F32, kind="ExternalInput").ap()
    outt = nc.dram_tensor("outt", [D, N], F32, kind="ExternalOutput").ap()

    with tile.TileContext(nc) as tc:
        _emit(nc, tc, xt, wq, wk, wv, wo, w1, w2, bo, b1, b2,
              ln1w, ln1b, ln2w, ln2b, outt)
    nc.compile()
    return nc


def _tiles(pool, n, shape, dt, tag):
    return [
        pool.tile(shape, dt, tag=f"{tag}{i}", name=f"{tag}{i}") for i in range(n)
    ]


def _emit(nc, tc, xt, wq, wk, wv, wo, w1, w2, bo, b1, b2,
          ln1w, ln1b, ln2w, ln2b, outt):
    with (
        tc.tile_pool(name="consts", bufs=1) as consts,
        tc.tile_pool(name="rows", bufs=2) as rows,
        tc.tile_pool(name="stage", bufs=3) as stage,
        tc.tile_pool(name="xmid", bufs=1) as xmid_pool,
        tc.tile_pool(name="mmps", bufs=4, space="PSUM") as mmps,
        tc.tile_pool(name="smps", bufs=1, space="PSUM") as smps,
    ):
        # ---- constants ----
        ones_col = consts.tile([P, 1], MM_SB, tag="onescol")
        nc.gpsimd.memset(ones_col[:], 1.0)
        ones_row = consts.tile([1, P], MM_SB, tag="onesrow")
        nc.gpsimd.memset(ones_row[:], 1.0)

        # per-partition columns: [128, DT] views of the 1-D vectors
        def col_load(vec, nt, tag):
            t = consts.tile([P, nt], F32, tag=tag)
            nc.sync.dma_start(t[:], vec.rearrange("(t p) -> p t", p=P))
            return t

        ln1w_c = col_load(ln1w, DT, "ln1wc")
        ln1b_c = col_load(ln1b, DT, "ln1bc")
        ln2w_c = col_load(ln2w, DT, "ln2wc")
        ln2b_c = col_load(ln2b, DT, "ln2bc")
        bo_c = col_load(bo, DT, "boc")
        b2_c = col_load(b2, DT, "b2c")
        b1_c = col_load(b1, FT, "b1c")

        # LN weight rows in matmul dtype (lhsT of the rank-1 broadcast)
        def row_load(vec, tag):
            st = consts.tile([1, D], F32, tag=tag + "f")
            nc.sync.dma_start(st[:], vec[None, :])
            t = consts.tile([1, D], MM_SB, tag=tag)
            nc.vector.tensor_copy(t[:], st[:])
            return t

        ln1w_r = row_load(ln1w, "ln1wr")
        ln2w_r = row_load(ln2w, "ln2wr")

        xmid = _tiles(xmid_pool, DT, [P, N], F32, "xm")

        def layernorm(src_tiles, src_mm_tiles, w_col, b_col, w_row, out_tiles):
            """out = w * (src - mean_over_dim(src)) + b, all transposed.

            src_tiles: DT x [P, N] fp32; src_mm_tiles: same data in MM dtype
            (used for the PE column-sum); out_tiles: DT x [P, N] MM dtype.
            """
            negmu = rows.tile([1, N], MM_SB, tag="negmu")
            for ch in range(NC):
                sps = smps.tile([1, NCH], F32, tag="sums")
                for k in range(DT):
                    nc.tensor.matmul(
                        sps[:], _mm(ones_col[:]),
                        _mm(src_mm_tiles[k][:, ts(ch, NCH)]),
                        start=(k == 0), stop=(k == DT - 1),
                    )
                # negated mean in matmul dtype
                nc.scalar.activation(negmu[:, ts(ch, NCH)], sps[:],
                                     AF.Copy, scale=-1.0 / D)
            for j in range(DT):
                for ch in range(NC):
                    # rank-1: (-mu) * w  broadcast to [128, NCH]
                    bps = smps.tile([P, NCH], F32, tag="bcast", bufs=2)
                    nc.tensor.matmul(bps[:], _mm(w_row[:, ts(j, P)]),
                                     _mm(negmu[:, ts(ch, NCH)]),
                                     start=True, stop=True)
                    t1 = stage.tile([P, NCH], F32, tag="lnt1")
                    nc.vector.tensor_scalar(
                        out=t1[:], in0=src_tiles[j][:, ts(ch, NCH)],
                        scalar1=w_col[:, j:j + 1], scalar2=b_col[:, j:j + 1],
                        op0=ALU.mult, op1=ALU.add)
                    nc.vector.tensor_tensor(
                        out=out_tiles[j][:, ts(ch, NCH)], in0=t1[:],
                        in1=bps[:], op=ALU.add)

        with (
            tc.tile_pool(name="xts", bufs=1) as xts_pool,
            tc.tile_pool(name="qkv", bufs=1) as qkv_pool,
            tc.tile_pool(name="aot", bufs=1) as aot_pool,
            tc.tile_pool(name="wo", bufs=1) as wo_pool,
        ):
            xts = _tiles(xts_pool, DT, [P, N], F32, "xt")
            for j in range(DT):
                nc.sync.dma_start(xts[j][:], xt[ts(j, P), :])

            qbf = _tiles(qkv_pool, DT, [P, N], MM_SB, "q")
            kbf = _tiles(qkv_pool, DT, [P, N], MM_SB, "k")
            vaug = _tiles(qkv_pool, NT, [P, H, DH + 1], MM_SB, "v")
            aot = _tiles(aot_pool, DT, [P, N], MM_SB, "ao")

            with (
                tc.tile_pool(name="ln1", bufs=1) as ln1_pool,
                tc.tile_pool(name="wqkv", bufs=1) as wqkv_pool,
            ):
                # ---- phase 0: LN1 ----
                xbf = _tiles(ln1_pool, DT, [P, N], MM_SB, "xb")
                for j in range(DT):
                    nc.vector.tensor_copy(xbf[j][:], xts[j][:])
                hbf = _tiles(ln1_pool, DT, [P, N], MM_SB, "h")
                layernorm(xts, xbf, ln1w_c, ln1b_c, ln1w_r, hbf)

                # ---- phase 1: QKV ----
                # wq/wk as M-slabs [P, DT, P]; wv natural [P, D] (rhs)
                wq_sb = _tiles(wqkv_pool, DT, [P, DT, P], MM_SB, "wq")
                wk_sb = _tiles(wqkv_pool, DT, [P, DT, P], MM_SB, "wk")
                wv_sb = _tiles(wqkv_pool, DT, [P, D], MM_SB, "wv")
                for m in range(DT):
                    nc.sync.dma_start(
                        wq_sb[m][:],
                        wq[:, ts(m, P)].rearrange("(t p) m -> p t m", p=P))
                    nc.sync.dma_start(
                        wk_sb[m][:],
                        wk[:, ts(m, P)].rearrange("(t p) m -> p t m", p=P))
                    nc.sync.dma_start(wv_sb[m][:], wv[ts(m, P), :])

                for m in range(DT):
                    for ch in range(NC):
                        qps = mmps.tile([P, NCH], F32, tag="mm")
                        for k in range(DT):
                            nc.tensor.matmul(
                                qps[:], _mm(wq_sb[m][:, k, :]),
                                _mm(hbf[k][:, ts(ch, NCH)]),
                                start=(k == 0), stop=(k == DT - 1))
                        nc.vector.tensor_copy(qbf[m][:, ts(ch, NCH)], qps[:])
                        kps = mmps.tile([P, NCH], F32, tag="mm")
                        for k in range(DT):
                            nc.tensor.matmul(
                                kps[:], _mm(wk_sb[m][:, k, :]),
                                _mm(hbf[k][:, ts(ch, NCH)]),
                                start=(k == 0), stop=(k == DT - 1))
                        nc.vector.tensor_copy(kbf[m][:, ts(ch, NCH)], kps[:])

                # V in [seq, inner] layout, head-scattered with ones column
                for i in range(NT):
                    nc.gpsimd.memset(vaug[i][:, :, DH:DH + 1], 1.0)
                    for ci, (c0, cw) in enumerate(((0, NCH), (NCH, D - NCH))):
                        vps = mmps.tile([P, NCH], F32, tag="mm")
                        for k in range(DT):
                            nc.tensor.matmul(
                                vps[:, :cw], _mm(hbf[k][:, ts(i, P)]),
                                _mm(wv_sb[k][:, ds(c0, cw)]),
                                start=(k == 0), stop=(k == DT - 1))
                        nh = cw // DH
                        nc.vector.tensor_copy(
                            vaug[i][:, ds(c0 // DH, nh), 0:DH],
                            vps[:, :cw].rearrange("p (h d) -> p h d", d=DH))

            # ---- phase 2: attention, head by head ----
            with tc.tile_pool(name="exp", bufs=1) as exp_pool:
                # 3 heads of exp tiles in flight
                exp_tiles = [
                    _tiles(exp_pool, NT, [P, N], MM_SB, f"e{s}")
                    for s in range(3)
                ]
                for h in range(H):
                    jt, off = h // 2, (h % 2) * DH
                    et = exp_tiles[h % 3]
                    for mi in range(NT):
                        for ch in range(NC):
                            dps = mmps.tile([P, NCH], F32, tag="mm")
                            nc.tensor.matmul(
                                dps[:],
                                _mm(kbf[jt][off:off + DH, ts(mi, P)]),
                                _mm(qbf[jt][off:off + DH, ts(ch, NCH)]),
                                start=True, stop=True)
                            nc.scalar.activation(
                                et[mi][:, ts(ch, NCH)], dps[:],
                                AF.Exp, scale=SCALE)
                    for ch in range(NC):
                        ops = smps.tile([DH + 1, NCH], F32, tag="ops")
                        for ki in range(NT):
                            nc.tensor.matmul(
                                ops[:], _mm(vaug[ki][:, h, :]),
                                _mm(et[ki][:, ts(ch, NCH)]),
                                start=(ki == 0), stop=(ki == NT - 1))
                        rec = rows.tile([1, NCH], F32, tag="recf")
                        nc.vector.reciprocal(rec[:], ops[DH:DH + 1, :])
                        recm = rows.tile([1, NCH], MM_SB, tag="recm")
                        nc.vector.tensor_copy(recm[:], rec[:])
                        bps = smps.tile([DH, NCH], F32, tag="bcast", bufs=2)
                        nc.tensor.matmul(bps[:], _mm(ones_row[:, :DH]),
                                         _mm(recm[:]), start=True, stop=True)
                        bsb = stage.tile([DH, NCH], F32, tag="bsb")
                        nc.vector.tensor_copy(bsb[:], bps[:])
                        nc.vector.tensor_tensor(
                            out=aot[jt][off:off + DH, ts(ch, NCH)],
                            in0=ops[0:DH, :], in1=bsb[:], op=ALU.mult)

            # ---- phase 3: Wo + residual ----
            wo_sb = _tiles(wo_pool, DT, [P, DT, P], MM_SB, "wo")
            for m in range(DT):
                nc.sync.dma_start(
                    wo_sb[m][:],
                    wo[:, ts(m, P)].rearrange("(t p) m -> p t m", p=P))
            for m in range(DT):
                for ch in range(NC):
                    ps = mmps.tile([P, NCH], F32, tag="mm")
                    for k in range(DT):
                        nc.tensor.matmul(
                            ps[:], _mm(wo_sb[m][:, k, :]),
                            _mm(aot[k][:, ts(ch, NCH)]),
                            start=(k == 0), stop=(k == DT - 1))
                    t1 = stage.tile([P, NCH], F32, tag="wot1")
                    nc.vector.tensor_scalar(
                        out=t1[:], in0=ps[:], scalar1=bo_c[:, m:m + 1],
                        scalar2=None, op0=ALU.add)
                    nc.vector.tensor_tensor(
                        out=xmid[m][:, ts(ch, NCH)], in0=t1[:],
                        in1=xts[m][:, ts(ch, NCH)], op=ALU.add)

        # ---- phase 4: LN2 ----
        with (
            tc.tile_pool(name="ln2", bufs=1) as ln2_pool,
            tc.tile_pool(name="wff", bufs=1) as wff_pool,
            tc.tile_pool(name="ff1", bufs=1) as ff1_pool,
        ):
            xmbf = _tiles(ln2_pool, DT, [P, N], MM_SB, "xmb")
            for j in range(DT):
                nc.vector.tensor_copy(xmbf[j][:], xmid[j][:])
            h2bf = _tiles(ln2_pool, DT, [P, N], MM_SB, "h2")
            layernorm(xmid, xmbf, ln2w_c, ln2b_c, ln2w_r, h2bf)

            # ---- phase 5: FFN ----
            w1_sb = _tiles(wff_pool, FT, [P, DT, P], MM_SB, "w1")
            for mf in range(FT):
                nc.sync.dma_start(
                    w1_sb[mf][:],
                    w1[:, ts(mf, P)].rearrange("(t p) m -> p t m", p=P))
            w2_sb = _tiles(wff_pool, DT, [P, FT, P], MM_SB, "w2")
            for m in range(DT):
                nc.sync.dma_start(
                    w2_sb[m][:],
                    w2[:, ts(m, P)].rearrange("(t p) m -> p t m", p=P))

            ff1 = ff1_pool.tile([P, FT, NCH], MM_SB, tag="ff1")
            for ch in range(NC):
                for mf in range(FT):
                    ps = mmps.tile([P, NCH], F32, tag="mm")
                    for k in range(DT):
                        nc.tensor.matmul(
                            ps[:], _mm(w1_sb[mf][:, k, :]),
                            _mm(h2bf[k][:, ts(ch, NCH)]),
                            start=(k == 0), stop=(k == DT - 1))
                    nc.scalar.activation(ff1[:, mf, :], ps[:], AF.Relu,
                                         bias=b1_c[:, mf:mf + 1])
                for m in range(DT):
                    ps = mmps.tile([P, NCH], F32, tag="mm")
                    for kf in range(FT):
                        nc.tensor.matmul(
                            ps[:], _mm(w2_sb[m][:, kf, :]),
                            _mm(ff1[:, kf, :]),
                            start=(kf == 0), stop=(kf == FT - 1))
                    t1 = stage.tile([P, NCH], F32, tag="fft1")
                    nc.vector.tensor_scalar(
                        out=t1[:], in0=ps[:], scalar1=b2_c[:, m:m + 1],
                        scalar2=None, op0=ALU.add)
                    ot = stage.tile([P, NCH], F32, tag="fot")
                    nc.vector.tensor_tensor(
                        out=ot[:], in0=t1[:], in1=xmid[m][:, ts(ch, NCH)],
                        op=ALU.add)
                    nc.sync.dma_start(outt[ts(m, P), ts(ch, NCH)], ot[:])


_CACHED = None


def _get_program():
    global _CACHED
    if _CACHED is None:
        _CACHED = build_program()
    return _CACHED


def prepare_in_maps(inputs):
    x = np.asarray(inputs["x"], dtype=np.float32)
    wcast = lambda a: np.ascontiguousarray(np.asarray(a, np.float32)).astype(MM_NP)
    f32c = lambda a: np.ascontiguousarray(np.asarray(a, np.float32))
    shared = {
        "wq": wcast(inputs["Wq"]), "wk": wcast(inputs["Wk"]),
        "wv": wcast(inputs["Wv"]), "wo": wcast(inputs["Wo"]),
        "w1": wcast(inputs["W1"]), "w2": wcast(inputs["W2"]),
        "bo": f32c(inputs["bo"]), "b1": f32c(inputs["b1"]),
        "b2": f32c(inputs["b2"]),
        "ln1w": f32c(inputs["ln1_w"]), "ln1b": f32c(inputs["ln1_b"]),
        "ln2w": f32c(inputs["ln2_w"]), "ln2b": f32c(inputs["ln2_b"]),
    }
    in_maps = []
    for i in range(B):
        m = dict(shared)
        m["xt"] = np.ascontiguousarray(x[i].T)  # [D, N]
        in_maps.append(m)
    return in_maps


def kernel(**inputs):
    nc = _get_program()
    in_maps = prepare_in_maps(inputs)
    res = run_bass_kernel_spmd(nc, in_maps, list(range(B)))
    out = np.stack([np.ascontiguousarray(r["outt"].T) for r in res.results])
    return out.astype(np.float32)
